# revision 4
# baseline (speedup 1.0000x reference)
"""Lambda-returns (GammaLambdaLearner) Trainium2 Bass kernel.

ret[t] = r[t] + gamma*(1-d[t]) * ((1-lam[t])*v[t+1] + lam[t]*ret[t+1]),
ret[S] = v[S]  -- a first-order linear recurrence in reversed time:
    ret[t] = a[t]*ret[t+1] + b[t]
    a[t] = gamma*(1-d[t])*lam[t]
    b[t] = r[t] + gamma*(1-d[t])*(1-lam[t])*v[t+1]

Mapping: batch on SBUF partitions, time on the free axis, host-flipped so the
hardware TensorTensorScan (state = a*state + b, forward along free dim) computes
the reversed-time recurrence directly.  Each partition row holds SEQS=32
consecutive batch elements' time series concatenated (a free host reshape that
gives 16KB+ DMA lines).  Cross-sequence leakage through the scan is cut by
zeroing a[] at every sequence-start column (baked into the glam param tile);
the bootstrap ret[S-1] = r + gamma*(1-d)*v[S] is obtained by setting the goml
param to -gamma at those columns, so the scan needs no per-sequence initial.
Pure data parallelism over 8 cores.
"""

import numpy as np
from contextlib import ExitStack

try:
    import concourse.bass as bass  # noqa: F401
except ImportError:  # pragma: no cover
    import sys

    sys.path.insert(0, "/opt/trn_rl_repo")

import concourse.bass as bass
import concourse.tile as tile
from concourse import bacc, mybir
from concourse.bass_utils import run_bass_kernel_spmd

B, S = 32768, 512
NCORES = 8
BL = B // NCORES  # 4096 batch rows per core
P = 128  # SBUF partitions
SEQS = BL // P  # 32 sequences concatenated per partition row
ROWLEN = SEQS * S  # 16384 elements per partition row
CH = 4  # sequences per compute tile
CW = CH * S  # 2048 free elements per compute tile
NG = SEQS // CH  # 8 tile groups per core
EPS = 1e-8

F32 = mybir.dt.float32
U8 = mybir.dt.uint8
_cached = {}


def _build_nc():
    nc = bacc.Bacc(
        "TRN2",
        target_bir_lowering=False,
        debug=False,
        enable_asserts=False,
        num_devices=NCORES,
    )
    d_in = nc.dram_tensor("d_rev", [P, ROWLEN], U8, kind="ExternalInput").ap()
    r_in = nc.dram_tensor("r_rev", [P, ROWLEN], F32, kind="ExternalInput").ap()
    v_in = nc.dram_tensor("v_rev", [P, ROWLEN], F32, kind="ExternalInput").ap()
    glam_in = nc.dram_tensor("glam_m", [P, CW], F32, kind="ExternalInput").ap()
    goml_in = nc.dram_tensor("goml_m", [P, CW], F32, kind="ExternalInput").ap()
    out = nc.dram_tensor("out_rev", [P, ROWLEN], F32, kind="ExternalOutput").ap()

    MULT = mybir.AluOpType.mult
    ADD = mybir.AluOpType.add

    with tile.TileContext(nc) as tc, ExitStack() as ctx:
        const_pool = ctx.enter_context(tc.tile_pool(name="const", bufs=1))
        in_pool = ctx.enter_context(tc.tile_pool(name="inp", bufs=3))
        tmp_pool = ctx.enter_context(tc.tile_pool(name="tmp", bufs=2))

        glam = const_pool.tile([P, CW], F32)
        nc.scalar.dma_start(glam[:], glam_in[:, :])
        goml = const_pool.tile([P, CW], F32)
        nc.sync.dma_start(goml[:], goml_in[:, :])

        for g in range(NG):
            cols = slice(g * CW, (g + 1) * CW)
            r_t = in_pool.tile([P, CW], F32)
            nc.scalar.dma_start(r_t[:], r_in[:, cols])
            v_t = in_pool.tile([P, CW], F32)
            nc.sync.dma_start(v_t[:], v_in[:, cols])
            d_t = in_pool.tile([P, CW], U8)
            nc.gpsimd.dma_start(d_t[:], d_in[:, cols])

            # e = d - 1  (Act engine: Copy(d*1 + (-1)), u8 -> f32)
            e_t = tmp_pool.tile([P, CW], F32)
            nc.scalar.activation(
                e_t[:], d_t[:], mybir.ActivationFunctionType.Copy, bias=-1.0
            )
            # a = (d-1) * (-gamma*lam) = gamma*(1-d)*lam; 0 at seq starts
            a_t = tmp_pool.tile([P, CW], F32)
            nc.vector.tensor_tensor(a_t[:], e_t[:], glam[:], MULT)
            # w = (d-1) * v1 = -(1-d)*v1
            w_t = tmp_pool.tile([P, CW], F32)
            nc.gpsimd.tensor_tensor(w_t[:], e_t[:], v_t[:], MULT)
            # t = w * (-gamma*(1-lam)) ; at seq starts w * (-gamma)
            t_t = tmp_pool.tile([P, CW], F32)
            nc.gpsimd.tensor_tensor(t_t[:], w_t[:], goml[:], MULT)
            # b = t + r
            b_t = tmp_pool.tile([P, CW], F32)
            nc.vector.tensor_tensor(b_t[:], t_t[:], r_t[:], ADD)
            # scan: state = a*state + b along free dim; a=0 at each seq start
            o_t = tmp_pool.tile([P, CW], F32)
            nc.vector.tensor_tensor_scan(o_t[:], a_t[:], b_t[:], 0.0, MULT, ADD)
            nc.gpsimd.dma_start(out[:, cols], o_t[:])

    nc.compile()
    return nc


def _get_nc():
    if "nc" not in _cached:
        _cached["nc"] = _build_nc()
    return _cached["nc"]


def _prep(values, rewards, dones, raw_gamma, raw_lambd):
    gamma = max(float(np.tanh(np.float32(raw_gamma[0]))), EPS)
    lam = np.maximum(np.tanh(raw_lambd.astype(np.float32)), EPS)  # [S]
    lam_rev = lam[::-1].copy()
    glam_row = np.tile(-gamma * lam_rev, CH).astype(np.float32)
    glam_row[0::S] = 0.0  # cut scan carry across concatenated sequences
    goml_row = np.tile(-gamma * (1.0 - lam_rev), CH).astype(np.float32)
    goml_row[0::S] = -gamma  # bootstrap: ret[S-1] = r + gamma*(1-d)*v[S]
    glam_m = np.ascontiguousarray(np.broadcast_to(glam_row[None, :], (P, CW)))
    goml_m = np.ascontiguousarray(np.broadcast_to(goml_row[None, :], (P, CW)))

    d_rev = np.ascontiguousarray(dones.reshape(B, S)[:, ::-1]).astype(np.uint8)
    r_rev = np.ascontiguousarray(rewards.reshape(B, S)[:, ::-1], dtype=np.float32)
    v_rev = np.ascontiguousarray(
        values.reshape(B, S + 1)[:, 1:][:, ::-1], dtype=np.float32
    )

    in_maps = []
    for c in range(NCORES):
        sl = slice(c * BL, (c + 1) * BL)
        in_maps.append(
            {
                "d_rev": d_rev[sl].reshape(P, ROWLEN),
                "r_rev": r_rev[sl].reshape(P, ROWLEN),
                "v_rev": v_rev[sl].reshape(P, ROWLEN),
                "glam_m": glam_m,
                "goml_m": goml_m,
            }
        )
    return in_maps


def kernel(values, rewards, dones, raw_gamma, raw_lambd, _trace=False):
    nc = _get_nc()
    in_maps = _prep(values, rewards, dones, raw_gamma, raw_lambd)
    res = run_bass_kernel_spmd(nc, in_maps, list(range(NCORES)), trace=_trace)
    if _trace:
        _cached["last_results"] = res
    out = np.empty((B, S), dtype=np.float32)
    for c in range(NCORES):
        out[c * BL : (c + 1) * BL] = res.results[c]["out_rev"].reshape(BL, S)[:, ::-1]
    return out.reshape(B, S, 1)
